# revision 1
# baseline (speedup 1.0000x reference)
"""Trainium2 Bass kernel for nn_GatherModel (NNConv GNN message passing).

8-core SPMD, edge-parallel sharded by destination node block:
  - core k owns nodes [k*6250, (k+1)*6250) and all edges whose dst lies there
  - per-edge weight matrices W'_e (o-major) are built once on device (PE) and
    streamed from HBM each of the 6 message-passing steps
  - per-edge contraction msg = x_src @ W_e runs on the Vector engine via a
    custom fused multiply+prefix-scan DVE op, extracting per-o sums by
    strided prefix differences
  - scatter (segment-sum over dst) is a PE matmul against on-device-built
    one-hot window matrices; node update runs in transposed feature layout
  - each step ends with an 8-core AllGather of the updated node features
"""
import numpy as np

import concourse.bacc as bacc
import concourse.bass as bass
import concourse.mybir as mybir
import concourse.tile as tile
from concourse import bass_utils, dve_ops
from concourse.dve_spec import Spec, Src0, Src1, scan, AluOp, lower, _has_src1
from concourse.dve_uop import DveOpSpec

N = 50000
E = 150000
D_IN = 42
D_H = 42
E_IN = 10
E_H = 128
STEPS = 6
N_CORES = 8
NPC = N // N_CORES          # 6250 nodes per core
WIN = 128                   # scatter window (node block) size
N_WIN = (NPC + WIN - 1) // WIN  # 49 windows per core, last partial (106)
NW = D_H * D_H              # 1764
F32 = mybir.dt.float32
I32 = mybir.dt.int32


def _register_prefix_mac():
    name = "PREFIX_MAC_GNN"
    if name in dve_ops._SUB_OPCODE_FOR_NAME:
        return next(op for op in dve_ops.OPS if op.name == name)
    spec = Spec(
        body=scan(AluOp.ADD, Src0 * Src1),
        reference=lambda in0, in1, s0, s1, imm2: np.cumsum(
            (in0.astype(np.float32) * in1).reshape(in0.shape[0], -1), axis=-1
        ),
    )
    shas = {}
    row = dve_ops._CUSTOM_DVE_ROW_BASE + len(dve_ops.OPS)
    for ver in ("v3", "v4"):
        uops = lower(spec, ver=ver)
        shas[ver] = DveOpSpec(name=name, opcode=row, uops=uops,
                              rd1_en=_has_src1(spec)).sha(ver)
    op = dve_ops.DveOp(name, spec, subdim=False, uops_sha=shas)
    dve_ops.OPS.append(op)
    dve_ops._SUB_OPCODE_FOR_NAME[name] = row
    dve_ops.CUSTOM_DVE_SPECS[name] = spec
    return op


def _host_prep(n_feat, e_feat, src, dst):
    """Sort edges by dst, shard by dst block, pad each (core, window) edge run
    onto a shared slot grid so the tile->window map is identical on all cores."""
    order = np.argsort(dst, kind="stable")
    src_s, dst_s, ef_s = src[order], dst[order], e_feat[order]

    # per (core, window) counts
    core_e = dst_s // NPC
    loc = dst_s - core_e * NPC
    win_e = loc // WIN
    cnt = np.zeros((N_CORES, N_WIN), dtype=np.int64)
    np.add.at(cnt, (core_e, win_e), 1)

    slot_cnt = cnt.max(axis=0)                       # shared grid
    G = np.concatenate([[0], np.cumsum(slot_cnt)])   # window slot boundaries
    total = int(G[-1])
    T = (total + 127) // 128                         # edge tiles per core
    E_PAD = T * 128

    # per-core padded edge arrays
    src_pad = np.zeros((N_CORES, E_PAD), dtype=np.int32)
    dstrel_pad = np.full((N_CORES, E_PAD), -1.0, dtype=np.float32)
    ef_pad = np.zeros((N_CORES, E_PAD, E_IN), dtype=np.float32)

    # tile -> window band
    w0 = np.zeros(T, dtype=np.int64)       # first window overlapping tile t
    bw = np.zeros(T, dtype=np.int64)       # how many windows overlap tile t
    for t in range(T):
        lo, hi = t * 128, min((t + 1) * 128, total)
        wlo = int(np.searchsorted(G, lo, side="right") - 1)
        whi = int(np.searchsorted(G, max(hi - 1, lo), side="right") - 1)
        wlo, whi = min(wlo, N_WIN - 1), min(whi, N_WIN - 1)
        w0[t] = wlo
        bw[t] = whi - wlo + 1
    B_W = int(bw.max())

    # fill padded arrays: window w of core k occupies slots [G[w], G[w]+cnt[k,w])
    core_starts = np.searchsorted(core_e, np.arange(N_CORES))
    for k in range(N_CORES):
        base = core_starts[k]
        cw = np.concatenate([[0], np.cumsum(cnt[k])])
        for w in range(N_WIN):
            s0, s1 = int(base + cw[w]), int(base + cw[w + 1])
            g0 = int(G[w])
            n_e = s1 - s0
            src_pad[k, g0:g0 + n_e] = src_s[s0:s1]
            ef_pad[k, g0:g0 + n_e] = ef_s[s0:s1]
            # dst_rel relative to the band anchor of the edge's tile
            slots = np.arange(g0, g0 + n_e)
            dstrel_pad[k, g0:g0 + n_e] = (
                loc[s0:s1] - w0[slots // 128] * WIN).astype(np.float32)

    # scatter pair list (t, w) from actual overlap, and per-window tile ranges
    pairs = []
    for t in range(T):
        for j in range(int(bw[t])):
            w = int(w0[t]) + j
            if w < N_WIN:
                pairs.append((t, w))
    win_tiles = {w: [t for (t, ww) in pairs if ww == w] for w in range(N_WIN)}

    grid = dict(T=T, E_PAD=E_PAD, B_W=B_W, w0=w0, bw=bw, win_tiles=win_tiles)

    per_core = []
    for k in range(N_CORES):
        per_core.append(dict(
            e_featT=np.ascontiguousarray(ef_pad[k].T),           # [10, E_PAD]
            n_featT=np.ascontiguousarray(n_feat[k * NPC:(k + 1) * NPC].T),  # [42, NPC]
            src_idx=np.ascontiguousarray(src_pad[k].reshape(T, 128).T).astype(np.int32),  # [128, T]
            dst_rel=np.ascontiguousarray(dstrel_pad[k].reshape(T, 128).T),  # [128, T]
        ))
    return grid, per_core


def _build_program(grid):
    T, B_W = grid["T"], grid["B_W"]
    w0, bw, win_tiles = grid["w0"], grid["bw"], grid["win_tiles"]
    PREFIX_MAC = _register_prefix_mac()

    nc = bacc.Bacc("TRN2", target_bir_lowering=False, debug=False,
                   num_devices=N_CORES)

    # ---- kernel I/O ----
    e_featT = nc.dram_tensor("e_featT", [E_IN, grid["E_PAD"]], F32, kind="ExternalInput")
    n_featT = nc.dram_tensor("n_featT", [D_IN, NPC], F32, kind="ExternalInput")
    src_idx = nc.dram_tensor("src_idx", [128, T], I32, kind="ExternalInput")
    dst_rel = nc.dram_tensor("dst_rel", [128, T], F32, kind="ExternalInput")
    iota = nc.dram_tensor("iota", [128, B_W * WIN], F32, kind="ExternalInput")
    en1_w = nc.dram_tensor("en1_w", [E_IN, E_H], F32, kind="ExternalInput")
    en1_b = nc.dram_tensor("en1_b", [1, E_H], F32, kind="ExternalInput")
    en2_wp = nc.dram_tensor("en2_wp", [E_H, NW], F32, kind="ExternalInput")
    en2_bp = nc.dram_tensor("en2_bp", [1, NW], F32, kind="ExternalInput")
    b_r = nc.dram_tensor("b_r", [D_H, D_H], F32, kind="ExternalInput")
    lin0_wt = nc.dram_tensor("lin0_wt", [D_IN, D_H], F32, kind="ExternalInput")
    lin0_br = nc.dram_tensor("lin0_br", [1, D_H], F32, kind="ExternalInput")
    msgw_top = nc.dram_tensor("msgw_top", [D_H, D_H], F32, kind="ExternalInput")
    msgw_bot = nc.dram_tensor("msgw_bot", [D_H, D_H], F32, kind="ExternalInput")
    msgb_r = nc.dram_tensor("msgb_r", [1, D_H], F32, kind="ExternalInput")
    convb_r = nc.dram_tensor("convb_r", [1, D_H], F32, kind="ExternalInput")
    ident = nc.dram_tensor("ident", [D_H, D_H], F32, kind="ExternalInput")
    ones_r = nc.dram_tensor("ones_r", [1, 128], F32, kind="ExternalInput")
    y = nc.dram_tensor("y", [NPC, D_H], F32, kind="ExternalOutput")

    with tile.TileContext(nc) as tc:
        with (
            tc.tile_pool(name="const", bufs=1) as cpool,
            tc.tile_pool(name="dram", bufs=1, space="DRAM") as dram,
        ):
            # ---- persistent SBUF residents ----
            nfT_sb = cpool.tile([D_IN, NPC], F32)
            srci_sb = cpool.tile([128, T], I32)
            dstr_sb = cpool.tile([128, T], F32)
            iota_sb = cpool.tile([128, B_W * WIN], F32)
            en1w_sb = cpool.tile([E_IN, E_H], F32)
            en1b_sb = cpool.tile([1, E_H], F32)
            en2wp_sb = cpool.tile([E_H, NW], F32)
            en2bp_sb = cpool.tile([1, NW], F32)
            br_sb = cpool.tile([D_H, D_H], F32)
            lin0w_sb = cpool.tile([D_IN, D_H], F32)
            lin0b_sb = cpool.tile([1, D_H], F32)
            mwt_sb = cpool.tile([D_H, D_H], F32)
            mwb_sb = cpool.tile([D_H, D_H], F32)
            mb_sb = cpool.tile([1, D_H], F32)
            cvb_sb = cpool.tile([1, D_H], F32)
            id_sb = cpool.tile([D_H, D_H], F32)
            ones_sb = cpool.tile([1, 128], F32)
            outT_a = cpool.tile([D_H, NPC], F32)
            outT_b = cpool.tile([D_H, NPC], F32)
            pfx = cpool.tile([128, 1 + NW], F32)

            for sb, dr in [(nfT_sb, n_featT), (srci_sb, src_idx),
                           (dstr_sb, dst_rel), (iota_sb, iota), (en1w_sb, en1_w),
                           (en1b_sb, en1_b), (en2wp_sb, en2_wp), (en2bp_sb, en2_bp), (br_sb, b_r),
                           (lin0w_sb, lin0_wt), (lin0b_sb, lin0_br), (mwt_sb, msgw_top),
                           (mwb_sb, msgw_bot), (mb_sb, msgb_r), (cvb_sb, convb_r),
                           (id_sb, ident), (ones_sb, ones_r)]:
                nc.sync.dma_start(sb[:], dr[:])
            nc.gpsimd.memset(pfx[:, 0:1], 0.0)

            # ---- DRAM scratch ----
            w_dram = dram.tile([T * 128, NW], F32)
            cc_in = [dram.tile([NPC, D_H], F32, name=f"cc_in{i}") for i in range(2)]
            cc_out = [dram.tile([N, D_H], F32, name=f"cc_out{i}", addr_space="Shared")
                      for i in range(STEPS)]

            # =========== setup: build W' in HBM ===========
            ECH = 16  # e_feat tiles per SBUF chunk
            with (
                tc.tile_pool(name="su_sb", bufs=3) as su_sb,
                tc.tile_pool(name="su_e", bufs=2) as su_e,
                tc.tile_pool(name="su_bias", bufs=1) as su_bias,
                tc.tile_pool(name="su_ph", bufs=2, space="PSUM") as su_ph,
                tc.tile_pool(name="su_pw", bufs=1, space="PSUM") as su_pw,
            ):
                pw = [su_pw.tile([128, 512], F32, name=f"pw{j}") for j in range(4)]
                nsz = [512, 512, 512, NW - 3 * 512]
                # broadcast en2 bias across all 128 partitions, once
                bias_sb = su_bias.tile([128, NW], F32)
                for j in range(4):
                    o0 = j * 512
                    nc.tensor.matmul(pw[j][:, :nsz[j]], lhsT=ones_sb[:1, :],
                                     rhs=en2bp_sb[:, o0:o0 + nsz[j]],
                                     start=True, stop=True)
                    nc.scalar.copy(bias_sb[:, o0:o0 + nsz[j]], pw[j][:, :nsz[j]])
                e_ch = None
                for t in range(T):
                    if t % ECH == 0:
                        c0 = t * 128
                        c1 = min((t + ECH) * 128, grid["E_PAD"])
                        e_ch = su_e.tile([E_IN, ECH * 128], F32, name="e_ch")
                        nc.sync.dma_start(e_ch[:, :c1 - c0], e_featT[:, c0:c1])
                    ph = su_ph.tile([128, 128], F32, name="ph")
                    o = (t % ECH) * 128
                    nc.tensor.matmul(ph[:], lhsT=en1w_sb[:], rhs=e_ch[:, o:o + 128],
                                     start=True, stop=False)
                    nc.tensor.matmul(ph[:], lhsT=en1b_sb[:], rhs=ones_sb[:1, :],
                                     start=False, stop=True)
                    h_sb = su_sb.tile([128, 128], F32, name="h_sb")
                    nc.scalar.activation(h_sb[:], ph[:], mybir.ActivationFunctionType.Relu)
                    w_sb = su_sb.tile([128, NW], F32, name="w_sb")
                    for j in range(4):
                        o0 = j * 512
                        nc.tensor.matmul(pw[j][:, :nsz[j]], lhsT=h_sb[:],
                                         rhs=en2wp_sb[:, o0:o0 + nsz[j]],
                                         start=True, stop=True)
                        nc.vector.tensor_tensor(
                            out=w_sb[:, o0:o0 + nsz[j]], in0=pw[j][:, :nsz[j]],
                            in1=bias_sb[:, o0:o0 + nsz[j]], op=mybir.AluOpType.add)
                    nc.sync.dma_start(w_dram[t * 128:(t + 1) * 128, :], w_sb[:])

            # =========== step pools ===========
            with (
                tc.tile_pool(name="st_w", bufs=12) as p_w,
                tc.tile_pool(name="st_x", bufs=6) as p_x,
                tc.tile_pool(name="st_m", bufs=4) as p_m,
                tc.tile_pool(name="st_oh", bufs=4) as p_oh,
                tc.tile_pool(name="st_sm", bufs=3) as p_sm,
                tc.tile_pool(name="ps_ag", bufs=3, space="PSUM") as ps_ag,
                tc.tile_pool(name="ps_st", bufs=2, space="PSUM") as ps_st,
                tc.tile_pool(name="ps_up", bufs=2, space="PSUM") as ps_up,
                tc.tile_pool(name="ps_tr", bufs=1, space="PSUM") as ps_tr,
            ):
                def window_cols(w):
                    n0 = w * WIN
                    m = min(WIN, NPC - n0)
                    return n0, m

                def update_window(w, outT_cur, outT_new, aggr_ps, step):
                    """Window epilogue: finish aggr, relu, update matmul, transpose, DMA."""
                    n0, m = window_cols(w)
                    last = step == STEPS
                    # + out (identity residual into conv) and conv bias
                    nc.tensor.matmul(aggr_ps[:, :m], lhsT=id_sb[:],
                                     rhs=outT_cur[:, n0:n0 + m], start=False, stop=False)
                    nc.tensor.matmul(aggr_ps[:, :m], lhsT=cvb_sb[:],
                                     rhs=ones_sb[:1, :m], start=False, stop=True)
                    mT_sb = p_sm.tile([D_H, WIN], F32, name="mT_sb")
                    nc.scalar.activation(mT_sb[:, :m], aggr_ps[:, :m],
                                         mybir.ActivationFunctionType.Relu)
                    up = ps_up.tile([D_H, WIN], F32, name="up")
                    nc.tensor.matmul(up[:, :m], lhsT=mwt_sb[:], rhs=mT_sb[:, :m],
                                     start=True, stop=False)
                    nc.tensor.matmul(up[:, :m], lhsT=mwb_sb[:], rhs=outT_cur[:, n0:n0 + m],
                                     start=False, stop=False)
                    nc.tensor.matmul(up[:, :m], lhsT=mb_sb[:], rhs=ones_sb[:1, :m],
                                     start=False, stop=not last)
                    if last:
                        nc.tensor.matmul(up[:, :m], lhsT=id_sb[:], rhs=nfT_sb[:, n0:n0 + m],
                                         start=False, stop=True)
                    nc.scalar.copy(outT_new[:, n0:n0 + m], up[:, :m])
                    tr = ps_tr.tile([128, D_H], F32, name="tr")
                    nc.tensor.transpose(tr[:m, :], outT_new[:, n0:n0 + m], id_sb[:])
                    rows = p_sm.tile([128, D_H], F32, name="rows")
                    nc.scalar.copy(rows[:m, :], tr[:m, :])
                    if last:
                        nc.sync.dma_start(y[n0:n0 + m, :], rows[:m, :])
                    else:
                        nc.sync.dma_start(cc_in[step % 2][n0:n0 + m, :], rows[:m, :])

                def all_gather(step):
                    nc.gpsimd.collective_compute(
                        "AllGather", mybir.AluOpType.bypass,
                        replica_groups=[list(range(N_CORES))],
                        ins=[cc_in[step % 2].opt()], outs=[cc_out[step].opt()])

                # =========== lin0: out0 = relu(n_feat @ lin0_w + b) ===========
                for w in range(N_WIN):
                    n0, m = window_cols(w)
                    up = ps_up.tile([D_H, WIN], F32, name="up")
                    nc.tensor.matmul(up[:, :m], lhsT=lin0w_sb[:], rhs=nfT_sb[:, n0:n0 + m],
                                     start=True, stop=False)
                    nc.tensor.matmul(up[:, :m], lhsT=lin0b_sb[:], rhs=ones_sb[:1, :m],
                                     start=False, stop=True)
                    nc.scalar.activation(outT_a[:, n0:n0 + m], up[:, :m],
                                         mybir.ActivationFunctionType.Relu)
                    tr = ps_tr.tile([128, D_H], F32, name="tr")
                    nc.tensor.transpose(tr[:m, :], outT_a[:, n0:n0 + m], id_sb[:])
                    rows = p_sm.tile([128, D_H], F32, name="rows")
                    nc.scalar.copy(rows[:m, :], tr[:m, :])
                    nc.sync.dma_start(cc_in[0][n0:n0 + m, :], rows[:m, :])
                all_gather(0)

                # =========== message passing steps ===========
                for step in range(1, STEPS + 1):
                    outT_cur = outT_a if step % 2 == 1 else outT_b
                    outT_new = outT_b if step % 2 == 1 else outT_a
                    src_buf = cc_out[step - 1]
                    aggr_of = {}
                    for t in range(T):
                        x_g = p_x.tile([128, D_H], F32, name="x_g")
                        nc.gpsimd.indirect_dma_start(
                            out=x_g[:], out_offset=None, in_=src_buf[:],
                            in_offset=bass.IndirectOffsetOnAxis(
                                ap=srci_sb[:, t:t + 1], axis=0))
                        w_t = p_w.tile([128, NW], F32, name="w_t")
                        nc.sync.dma_start(w_t[:], w_dram[t * 128:(t + 1) * 128, :])
                        nc.vector._custom_dve(
                            PREFIX_MAC, out=pfx[:, 1:1 + NW], in0=w_t[:],
                            in1=x_g[:, None, :].to_broadcast([128, D_H, D_H]))
                        msg = p_m.tile([128, D_H], F32, name="msg")
                        nc.vector.tensor_tensor(
                            out=msg[:], in0=pfx[:, D_H:1 + NW:D_H],
                            in1=pfx[:, 0:NW:D_H], op=mybir.AluOpType.subtract)
                        bwt = int(bw[t])
                        oh = p_oh.tile([128, B_W * WIN], F32, name="oh")
                        nc.vector.tensor_scalar(
                            out=oh[:, :bwt * WIN], in0=iota_sb[:, :bwt * WIN],
                            scalar1=dstr_sb[:, t:t + 1],
                            scalar2=None, op0=mybir.AluOpType.is_equal)
                        # scatter matmuls
                        for j in range(bwt):
                            w = int(w0[t]) + j
                            if w >= N_WIN:
                                continue
                            tiles_w = win_tiles[w]
                            if w not in aggr_of:
                                aggr_of[w] = ps_ag.tile([D_H, WIN], F32, name="aggr")
                            first = t == tiles_w[0]
                            last_t = t == tiles_w[-1]
                            nc.tensor.matmul(aggr_of[w][:], lhsT=msg[:],
                                             rhs=oh[:, j * WIN:(j + 1) * WIN],
                                             start=first, stop=False)
                            if last_t:
                                update_window(w, outT_cur, outT_new,
                                              aggr_of.pop(w), step)
                    if step < STEPS:
                        all_gather(step)

    nc.compile()
    return nc


_CACHED = {}


def kernel(n_feat, e_feat, src, dst, lin0_w, lin0_b, en1_w, en1_b,
           en2_w, en2_b, conv_bias, msg_w, msg_b):
    n_feat = np.asarray(n_feat, dtype=np.float32)
    e_feat = np.asarray(e_feat, dtype=np.float32)
    src = np.asarray(src, dtype=np.int32)
    dst = np.asarray(dst, dtype=np.int32)

    grid, per_core = _host_prep(n_feat, e_feat, src, dst)

    key = (grid["T"], grid["B_W"], tuple(grid["w0"].tolist()))
    if key not in _CACHED:
        _CACHED.clear()
        _CACHED[key] = _build_program(grid)
    nc = _CACHED[key]

    en2_wp = np.ascontiguousarray(
        np.asarray(en2_w, np.float32).reshape(E_H, D_H, D_H).transpose(0, 2, 1).reshape(E_H, NW))
    shared = dict(
        iota=np.tile(np.arange(grid["B_W"] * WIN, dtype=np.float32), (128, 1)),
        en1_w=np.asarray(en1_w, np.float32),
        en1_b=np.asarray(en1_b, np.float32).reshape(1, E_H),
        en2_wp=en2_wp,
        en2_bp=np.ascontiguousarray(
            np.asarray(en2_b, np.float32).reshape(D_H, D_H).T.reshape(1, NW)),
        b_r=np.ascontiguousarray(np.asarray(en2_b, np.float32).reshape(D_H, D_H)),
        lin0_wt=np.asarray(lin0_w, np.float32),
        lin0_br=np.asarray(lin0_b, np.float32).reshape(1, D_H),
        msgw_top=np.ascontiguousarray(np.asarray(msg_w, np.float32)[:D_H, :]),
        msgw_bot=np.ascontiguousarray(np.asarray(msg_w, np.float32)[D_H:, :]),
        msgb_r=np.asarray(msg_b, np.float32).reshape(1, D_H),
        convb_r=np.asarray(conv_bias, np.float32).reshape(1, D_H),
        ident=np.eye(D_H, dtype=np.float32),
        ones_r=np.ones((1, 128), dtype=np.float32),
    )
    in_maps = []
    for k in range(N_CORES):
        m = dict(shared)
        m.update(per_core[k])
        in_maps.append(m)

    res = bass_utils.run_bass_kernel_spmd(nc, in_maps, core_ids=list(range(N_CORES)))
    out = np.concatenate([res.results[k]["y"] for k in range(N_CORES)], axis=0)
    return out.astype(np.float32)



# revision 7
# speedup vs baseline: 1.3771x; 1.3771x over previous
"""Trainium2 Bass kernel for nn_GatherModel (NNConv GNN message passing).

8-core SPMD, edge-parallel sharded by destination node block.

v2 design (vs baseline): the [E, 42, 42] per-edge weight tensor W' is never
materialized in HBM. Instead the small factor h_e = relu(e_feat @ en1_w + b)
([E,128] bf16) stays SBUF-resident and each step W' tiles are rebuilt on the
Tensor engine into PSUM (W' = h @ en2_w', step-invariant), consumed directly
by the Vector-engine multiply+prefix-scan contraction. This removes ~1 GB of
HBM traffic per core. The en2 bias is applied via a fused scatter: the
scatter matmul's stationary operand is [msg | x_src] so each window's PSUM
accumulates both the message aggregate and S = sum of source features, and
the epilogue adds S @ B with one small matmul.
  - scatter (segment-sum over dst) is a PE matmul against precomputed one-hot
    window matrices (SBUF-resident bf16, built once in setup)
  - node update runs in fp32 transposed feature layout; updated features are
    cast to bf16 rows and AllGather'd across the 8 cores each step
"""
import numpy as np

import concourse.bacc as bacc
import concourse.bass as bass
import concourse.mybir as mybir
import concourse.tile as tile
from concourse import bass_utils, dve_ops
from concourse.dve_spec import Spec, Src0, Src1, scan, AluOp, lower, _has_src1
from concourse.dve_uop import DveOpSpec

N = 50000
E = 150000
D_IN = 42
D_H = 42
E_IN = 10
E_H = 128
STEPS = 6
N_CORES = 8
NPC = N // N_CORES          # 6250 nodes per core
WIN = 128                   # scatter window (node block) size
N_WIN = (NPC + WIN - 1) // WIN  # 49 windows per core, last partial (106)
NW = D_H * D_H              # 1764
HNW = NW // 2               # 882 = 21 o-blocks of 42
O_HALF = D_H // 2           # 21
CHUNK = 441                 # rebuild matmul chunk (1 PSUM bank holds 512 fp32)
F32 = mybir.dt.float32
BF16 = mybir.dt.bfloat16
I32 = mybir.dt.int32


def _register_prefix_mac():
    name = "PREFIX_MAC_GNN"
    if name in dve_ops._SUB_OPCODE_FOR_NAME:
        return next(op for op in dve_ops.OPS if op.name == name)
    spec = Spec(
        body=scan(AluOp.ADD, Src0 * Src1),
        reference=lambda in0, in1, s0, s1, imm2: np.cumsum(
            (in0.astype(np.float32) * in1).reshape(in0.shape[0], -1), axis=-1
        ),
    )
    shas = {}
    row = dve_ops._CUSTOM_DVE_ROW_BASE + len(dve_ops.OPS)
    for ver in ("v3", "v4"):
        uops = lower(spec, ver=ver)
        shas[ver] = DveOpSpec(name=name, opcode=row, uops=uops,
                              rd1_en=_has_src1(spec)).sha(ver)
    op = dve_ops.DveOp(name, spec, subdim=False, uops_sha=shas)
    dve_ops.OPS.append(op)
    dve_ops._SUB_OPCODE_FOR_NAME[name] = row
    dve_ops.CUSTOM_DVE_SPECS[name] = spec
    return op


def _host_prep(n_feat, e_feat, src, dst):
    """Sort edges by dst, shard by dst block, pad each (core, window) edge run
    onto a shared slot grid so the tile->window map is identical on all cores."""
    order = np.argsort(dst, kind="stable")
    src_s, dst_s, ef_s = src[order], dst[order], e_feat[order]

    # per (core, window) counts
    core_e = dst_s // NPC
    loc = dst_s - core_e * NPC
    win_e = loc // WIN
    cnt = np.zeros((N_CORES, N_WIN), dtype=np.int64)
    np.add.at(cnt, (core_e, win_e), 1)

    slot_cnt = cnt.max(axis=0)                       # shared grid
    G = np.concatenate([[0], np.cumsum(slot_cnt)])   # window slot boundaries
    total = int(G[-1])
    T = (total + 127) // 128                         # edge tiles per core
    E_PAD = T * 128

    # per-core padded edge arrays
    src_pad = np.zeros((N_CORES, E_PAD), dtype=np.int32)
    dstrel_pad = np.full((N_CORES, E_PAD), -1.0, dtype=np.float32)
    ef_pad = np.zeros((N_CORES, E_PAD, E_IN), dtype=np.float32)

    # tile -> window band
    w0 = np.zeros(T, dtype=np.int64)       # first window overlapping tile t
    bw = np.zeros(T, dtype=np.int64)       # how many windows overlap tile t
    for t in range(T):
        lo, hi = t * 128, min((t + 1) * 128, total)
        wlo = int(np.searchsorted(G, lo, side="right") - 1)
        whi = int(np.searchsorted(G, max(hi - 1, lo), side="right") - 1)
        wlo, whi = min(wlo, N_WIN - 1), min(whi, N_WIN - 1)
        w0[t] = wlo
        bw[t] = whi - wlo + 1
    B_W = int(bw.max())

    # fill padded arrays: window w of core k occupies slots [G[w], G[w]+cnt[k,w])
    core_starts = np.searchsorted(core_e, np.arange(N_CORES))
    for k in range(N_CORES):
        base = core_starts[k]
        cw = np.concatenate([[0], np.cumsum(cnt[k])])
        for w in range(N_WIN):
            s0, s1 = int(base + cw[w]), int(base + cw[w + 1])
            g0 = int(G[w])
            n_e = s1 - s0
            src_pad[k, g0:g0 + n_e] = src_s[s0:s1]
            ef_pad[k, g0:g0 + n_e] = ef_s[s0:s1]
            # dst_rel relative to the band anchor of the edge's tile
            slots = np.arange(g0, g0 + n_e)
            dstrel_pad[k, g0:g0 + n_e] = (
                loc[s0:s1] - w0[slots // 128] * WIN).astype(np.float32)

    # scatter pair list (t, w) from actual overlap, and per-window tile ranges
    pairs = []
    for t in range(T):
        for j in range(int(bw[t])):
            w = int(w0[t]) + j
            if w < N_WIN:
                pairs.append((t, w))
    win_tiles = {w: [t for (t, ww) in pairs if ww == w] for w in range(N_WIN)}

    # offset of each tile's one-hot block inside the resident oh buffer
    oh_off = np.zeros(T + 1, dtype=np.int64)
    for t in range(T):
        oh_off[t + 1] = oh_off[t] + int(bw[t]) * WIN

    grid = dict(T=T, E_PAD=E_PAD, B_W=B_W, w0=w0, bw=bw, win_tiles=win_tiles,
                oh_off=oh_off)

    import ml_dtypes
    per_core = []
    for k in range(N_CORES):
        per_core.append(dict(
            e_featT=np.ascontiguousarray(ef_pad[k].T).astype(ml_dtypes.bfloat16),  # [10, E_PAD]
            n_featT=np.ascontiguousarray(n_feat[k * NPC:(k + 1) * NPC].T),  # [42, NPC]
            src_idx=np.ascontiguousarray(src_pad[k].reshape(T, 128).T).astype(np.int32),  # [128, T]
            dst_rel=np.ascontiguousarray(dstrel_pad[k].reshape(T, 128).T),  # [128, T]
        ))
    return grid, per_core


def _build_program(grid):
    T, B_W = grid["T"], grid["B_W"]
    w0, bw, win_tiles = grid["w0"], grid["bw"], grid["win_tiles"]
    oh_off = grid["oh_off"]
    OH_TOT = int(oh_off[T])
    PREFIX_MAC = _register_prefix_mac()

    nc = bacc.Bacc("TRN2", target_bir_lowering=False, debug=False,
                   num_devices=N_CORES)

    # ---- kernel I/O ----
    e_featT = nc.dram_tensor("e_featT", [E_IN, grid["E_PAD"]], BF16, kind="ExternalInput")
    n_featT = nc.dram_tensor("n_featT", [D_IN, NPC], F32, kind="ExternalInput")
    src_idx = nc.dram_tensor("src_idx", [128, T], I32, kind="ExternalInput")
    dst_rel = nc.dram_tensor("dst_rel", [128, T], F32, kind="ExternalInput")
    iota = nc.dram_tensor("iota", [128, B_W * WIN], F32, kind="ExternalInput")
    en1_w = nc.dram_tensor("en1_w", [E_IN, E_H], BF16, kind="ExternalInput")
    en1_b = nc.dram_tensor("en1_b", [1, E_H], BF16, kind="ExternalInput")
    en2_wp = nc.dram_tensor("en2_wp", [E_H, NW], BF16, kind="ExternalInput")
    b_r = nc.dram_tensor("b_r", [D_H, D_H], BF16, kind="ExternalInput")
    lin0_wt = nc.dram_tensor("lin0_wt", [D_IN, D_H], F32, kind="ExternalInput")
    lin0_br = nc.dram_tensor("lin0_br", [1, D_H], F32, kind="ExternalInput")
    msgw_top = nc.dram_tensor("msgw_top", [D_H, D_H], F32, kind="ExternalInput")
    msgw_bot = nc.dram_tensor("msgw_bot", [D_H, D_H], F32, kind="ExternalInput")
    msgb_r = nc.dram_tensor("msgb_r", [1, D_H], F32, kind="ExternalInput")
    convb_r = nc.dram_tensor("convb_r", [1, D_H], F32, kind="ExternalInput")
    ident = nc.dram_tensor("ident", [D_H, D_H], F32, kind="ExternalInput")
    ones_r = nc.dram_tensor("ones_r", [1, 128], F32, kind="ExternalInput")
    ones_b = nc.dram_tensor("ones_b", [1, 128], BF16, kind="ExternalInput")
    y = nc.dram_tensor("y", [NPC, D_H], F32, kind="ExternalOutput")

    with tile.TileContext(nc) as tc:
        with (
            tc.tile_pool(name="const", bufs=1) as cpool,
            tc.tile_pool(name="dram", bufs=1, space="DRAM") as dram,
        ):
            # ---- persistent SBUF residents ----
            nfT_sb = cpool.tile([D_IN, NPC], F32)
            srci_sb = cpool.tile([128, T], I32)
            dstr_sb = cpool.tile([128, T], F32)
            iota_sb = cpool.tile([128, B_W * WIN], F32)
            en1w_sb = cpool.tile([E_IN, E_H], BF16)
            en1b_sb = cpool.tile([1, E_H], BF16)
            en2wp_sb = cpool.tile([E_H, NW], BF16)
            br_sb = cpool.tile([D_H, D_H], BF16)
            lin0w_sb = cpool.tile([D_IN, D_H], F32)
            lin0b_sb = cpool.tile([1, D_H], F32)
            mwt_sb = cpool.tile([D_H, D_H], F32)
            mwb_sb = cpool.tile([D_H, D_H], F32)
            mb_sb = cpool.tile([1, D_H], F32)
            cvb_sb = cpool.tile([1, D_H], F32)
            id_sb = cpool.tile([D_H, D_H], F32)
            ones_sb = cpool.tile([1, 128], F32)
            onesb_sb = cpool.tile([1, 128], BF16)
            outT_a = cpool.tile([D_H, NPC], F32)
            outT_b = cpool.tile([D_H, NPC], F32)
            h_all = cpool.tile([128, T * 128], BF16)       # resident h^T tiles
            oh_all = cpool.tile([128, OH_TOT], BF16)       # resident one-hots
            # two prefix-scan halves, each: zero cell + 882 sums (stride 884)
            pfx = cpool.tile([128, 2, HNW + 2], F32)

            for sb, dr in [(nfT_sb, n_featT), (srci_sb, src_idx),
                           (dstr_sb, dst_rel), (iota_sb, iota), (en1w_sb, en1_w),
                           (en1b_sb, en1_b), (en2wp_sb, en2_wp), (br_sb, b_r),
                           (lin0w_sb, lin0_wt), (lin0b_sb, lin0_br), (mwt_sb, msgw_top),
                           (mwb_sb, msgw_bot), (mb_sb, msgb_r), (cvb_sb, convb_r),
                           (id_sb, ident), (ones_sb, ones_r), (onesb_sb, ones_b)]:
                nc.sync.dma_start(sb[:], dr[:])
            nc.gpsimd.memset(pfx[:, :, 0:1], 0.0)

            # ---- DRAM scratch ----
            cc_in = [dram.tile([NPC, D_H], BF16, name=f"cc_in{i}") for i in range(2)]
            cc_out = [dram.tile([N, D_H], BF16, name=f"cc_out{i}", addr_space="Shared")
                      for i in range(STEPS)]

            # =========== setup: h tiles, one-hot tiles, lin0 ===========
            ECH = 16  # e_feat tiles per SBUF chunk
            with (
                tc.tile_pool(name="su_e", bufs=2) as su_e,
                tc.tile_pool(name="su_ph", bufs=4, space="PSUM") as su_ph,
            ):
                e_ch = None
                for t in range(T):
                    if t % ECH == 0:
                        c0 = t * 128
                        c1 = min((t + ECH) * 128, grid["E_PAD"])
                        e_ch = su_e.tile([E_IN, ECH * 128], BF16, name="e_ch")
                        nc.sync.dma_start(e_ch[:, :c1 - c0], e_featT[:, c0:c1])
                    ph = su_ph.tile([128, 128], F32, name="ph")
                    o = (t % ECH) * 128
                    nc.tensor.matmul(ph[:], lhsT=en1w_sb[:], rhs=e_ch[:, o:o + 128],
                                     start=True, stop=False)
                    nc.tensor.matmul(ph[:], lhsT=en1b_sb[:], rhs=onesb_sb[:1, :],
                                     start=False, stop=True)
                    nc.scalar.activation(h_all[:, t * 128:(t + 1) * 128], ph[:],
                                         mybir.ActivationFunctionType.Relu)
                    # one-hot scatter block for this tile (static across steps)
                    bwt = int(bw[t])
                    o0 = int(oh_off[t])
                    nc.vector.tensor_scalar(
                        out=oh_all[:, o0:o0 + bwt * WIN],
                        in0=iota_sb[:, :bwt * WIN],
                        scalar1=dstr_sb[:, t:t + 1],
                        scalar2=None, op0=mybir.AluOpType.is_equal)

            # =========== step pools ===========
            # PSUM budget (8 banks): W pipeline 2x2 + per-window packed pair
            # 2x(1+1).  "agt" packs msg-aggregate + transpose scratch; "ast"
            # packs the source-feature sum S + the update matmul output.
            with (
                tc.tile_pool(name="st_m", bufs=6) as p_m,
                tc.tile_pool(name="st_sm", bufs=5) as p_sm,
                tc.tile_pool(name="ps_w", bufs=2, space="PSUM") as ps_w,
                tc.tile_pool(name="ps_win", bufs=2, space="PSUM") as ps_win,
            ):
                def window_cols(w):
                    n0 = w * WIN
                    m = min(WIN, NPC - n0)
                    return n0, m

                def new_window_tiles():
                    agt = ps_win.tile([128, 512], F32, name="agt")
                    ast = ps_win.tile([D_H, 512], F32, name="ast")
                    return agt, ast

                def update_window(w, outT_cur, outT_new, agt, ast, step):
                    """Window epilogue: S@B bias, residual, relu, update matmul,
                    transpose, DMA rows out."""
                    n0, m = window_cols(w)
                    last = step == STEPS
                    aggr = agt[0:D_H, 0:WIN]
                    s_ps = ast[:, 0:WIN]
                    up = ast[:, WIN:2 * WIN]
                    tr = agt[:, WIN:WIN + D_H]
                    # en2 bias via aggregated source features: aggr += (S @ B)^T
                    s_sb = p_sm.tile([D_H, WIN], BF16, name="s_sb")
                    nc.scalar.copy(s_sb[:, :m], s_ps[:, :m])
                    nc.tensor.matmul(aggr[:, :m], lhsT=br_sb[:],
                                     rhs=s_sb[:, :m], start=False, stop=False)
                    # + out (identity residual into conv) and conv bias
                    nc.tensor.matmul(aggr[:, :m], lhsT=id_sb[:],
                                     rhs=outT_cur[:, n0:n0 + m], start=False, stop=False)
                    nc.tensor.matmul(aggr[:, :m], lhsT=cvb_sb[:],
                                     rhs=ones_sb[:1, :m], start=False, stop=True)
                    mT_sb = p_sm.tile([D_H, WIN], F32, name="mT_sb")
                    nc.scalar.activation(mT_sb[:, :m], aggr[:, :m],
                                         mybir.ActivationFunctionType.Relu)
                    nc.tensor.matmul(up[:, :m], lhsT=mwt_sb[:], rhs=mT_sb[:, :m],
                                     start=True, stop=False)
                    nc.tensor.matmul(up[:, :m], lhsT=mwb_sb[:], rhs=outT_cur[:, n0:n0 + m],
                                     start=False, stop=False)
                    nc.tensor.matmul(up[:, :m], lhsT=mb_sb[:], rhs=ones_sb[:1, :m],
                                     start=False, stop=not last)
                    if last:
                        nc.tensor.matmul(up[:, :m], lhsT=id_sb[:], rhs=nfT_sb[:, n0:n0 + m],
                                         start=False, stop=True)
                    nc.scalar.copy(outT_new[:, n0:n0 + m], up[:, :m])
                    nc.tensor.transpose(tr[:m, :], outT_new[:, n0:n0 + m], id_sb[:])
                    if last:
                        rows = p_sm.tile([128, D_H], F32, name="rows_f")
                        nc.scalar.copy(rows[:m, :], tr[:m, :])
                        nc.sync.dma_start(y[n0:n0 + m, :], rows[:m, :])
                    else:
                        rows = p_sm.tile([128, D_H], BF16, name="rows")
                        nc.scalar.copy(rows[:m, :], tr[:m, :])
                        nc.sync.dma_start(cc_in[step % 2][n0:n0 + m, :], rows[:m, :])

                def all_gather(step):
                    nc.gpsimd.collective_compute(
                        "AllGather", mybir.AluOpType.bypass,
                        replica_groups=[list(range(N_CORES))],
                        ins=[cc_in[step % 2].opt()], outs=[cc_out[step].opt()])

                # =========== lin0: out0 = relu(n_feat @ lin0_w + b) ===========
                for w in range(N_WIN):
                    n0, m = window_cols(w)
                    agt, ast = new_window_tiles()
                    up = ast[:, WIN:2 * WIN]
                    tr = agt[:, WIN:WIN + D_H]
                    nc.tensor.matmul(up[:, :m], lhsT=lin0w_sb[:], rhs=nfT_sb[:, n0:n0 + m],
                                     start=True, stop=False)
                    nc.tensor.matmul(up[:, :m], lhsT=lin0b_sb[:], rhs=ones_sb[:1, :m],
                                     start=False, stop=True)
                    nc.scalar.activation(outT_a[:, n0:n0 + m], up[:, :m],
                                         mybir.ActivationFunctionType.Relu)
                    nc.tensor.transpose(tr[:m, :], outT_a[:, n0:n0 + m], id_sb[:])
                    rows = p_sm.tile([128, D_H], BF16, name="rows")
                    nc.scalar.copy(rows[:m, :], tr[:m, :])
                    nc.sync.dma_start(cc_in[0][n0:n0 + m, :], rows[:m, :])
                all_gather(0)

                # =========== message passing steps ===========
                for step in range(1, STEPS + 1):
                    outT_cur = outT_a if step % 2 == 1 else outT_b
                    outT_new = outT_b if step % 2 == 1 else outT_a
                    src_buf = cc_out[step - 1]
                    aggr_of = {}
                    for t in range(T):
                        h_t = h_all[:, t * 128:(t + 1) * 128]
                        # gathered src feats live in cols [42:84) of the
                        # scatter stationary [msg | x]
                        mx = p_m.tile([128, 2 * D_H], BF16, name="mx")
                        nc.gpsimd.indirect_dma_start(
                            out=mx[:, D_H:2 * D_H], out_offset=None, in_=src_buf[:],
                            in_offset=bass.IndirectOffsetOnAxis(
                                ap=srci_sb[:, t:t + 1], axis=0))
                        x_bc = mx[:, D_H:2 * D_H][:, None, :].to_broadcast(
                            [128, O_HALF, D_H])
                        # rebuild W' = h @ en2_w' on PE, half a tile at a time;
                        # DVE contracts straight out of PSUM
                        for hf in range(2):
                            wp = ps_w.tile([128, 2, 512], F32, name="wp")
                            for c in range(2):
                                c0 = (2 * hf + c) * CHUNK
                                nc.tensor.matmul(wp[:, c, :CHUNK], lhsT=h_t,
                                                 rhs=en2wp_sb[:, c0:c0 + CHUNK],
                                                 start=True, stop=True)
                            nc.vector._custom_dve(
                                PREFIX_MAC, out=pfx[:, hf, 1:HNW + 1],
                                in0=wp[:, :, :CHUNK], in1=x_bc)
                        nc.vector.tensor_tensor(
                            out=mx[:, 0:D_H].rearrange("p (h o) -> p h o", h=2),
                            in0=pfx[:, :, D_H:HNW + 1:D_H],
                            in1=pfx[:, :, 0:HNW - D_H + 1:D_H],
                            op=mybir.AluOpType.subtract)
                        # scatter matmuls: message aggregate + source-feat sum S
                        bwt = int(bw[t])
                        o0 = int(oh_off[t])
                        for j in range(bwt):
                            w = int(w0[t]) + j
                            if w >= N_WIN:
                                continue
                            tiles_w = win_tiles[w]
                            if w not in aggr_of:
                                aggr_of[w] = new_window_tiles()
                            first = t == tiles_w[0]
                            last_t = t == tiles_w[-1]
                            oh_j = oh_all[:, o0 + j * WIN:o0 + (j + 1) * WIN]
                            agt, ast = aggr_of[w]
                            nc.tensor.matmul(agt[0:D_H, 0:WIN], lhsT=mx[:, 0:D_H],
                                             rhs=oh_j, start=first, stop=False)
                            nc.tensor.matmul(ast[:, 0:WIN], lhsT=mx[:, D_H:2 * D_H],
                                             rhs=oh_j, start=first, stop=last_t)
                            if last_t:
                                update_window(w, outT_cur, outT_new,
                                              agt, ast, step)
                                aggr_of.pop(w)
                    if step < STEPS:
                        all_gather(step)

    nc.compile()
    return nc


_CACHED = {}


def kernel(n_feat, e_feat, src, dst, lin0_w, lin0_b, en1_w, en1_b,
           en2_w, en2_b, conv_bias, msg_w, msg_b):
    import ml_dtypes
    n_feat = np.asarray(n_feat, dtype=np.float32)
    e_feat = np.asarray(e_feat, dtype=np.float32)
    src = np.asarray(src, dtype=np.int32)
    dst = np.asarray(dst, dtype=np.int32)

    grid, per_core = _host_prep(n_feat, e_feat, src, dst)

    key = (grid["T"], grid["B_W"], tuple(grid["w0"].tolist()))
    if key not in _CACHED:
        _CACHED.clear()
        _CACHED[key] = _build_program(grid)
    nc = _CACHED[key]

    # en2_w reshaped so W' columns are (o, i) o-major, matching the scan's
    # per-o prefix-difference extraction
    en2_wp = np.ascontiguousarray(
        np.asarray(en2_w, np.float32).reshape(E_H, D_H, D_H).transpose(0, 2, 1)
        .reshape(E_H, NW)).astype(ml_dtypes.bfloat16)
    shared = dict(
        iota=np.tile(np.arange(grid["B_W"] * WIN, dtype=np.float32), (128, 1)),
        en1_w=np.asarray(en1_w, np.float32).astype(ml_dtypes.bfloat16),
        en1_b=np.asarray(en1_b, np.float32).reshape(1, E_H).astype(ml_dtypes.bfloat16),
        en2_wp=en2_wp,
        b_r=np.ascontiguousarray(
            np.asarray(en2_b, np.float32).reshape(D_H, D_H)).astype(ml_dtypes.bfloat16),
        lin0_wt=np.asarray(lin0_w, np.float32),
        lin0_br=np.asarray(lin0_b, np.float32).reshape(1, D_H),
        msgw_top=np.ascontiguousarray(np.asarray(msg_w, np.float32)[:D_H, :]),
        msgw_bot=np.ascontiguousarray(np.asarray(msg_w, np.float32)[D_H:, :]),
        msgb_r=np.asarray(msg_b, np.float32).reshape(1, D_H),
        convb_r=np.asarray(conv_bias, np.float32).reshape(1, D_H),
        ident=np.eye(D_H, dtype=np.float32),
        ones_r=np.ones((1, 128), dtype=np.float32),
        ones_b=np.ones((1, 128), dtype=ml_dtypes.bfloat16),
    )
    in_maps = []
    for k in range(N_CORES):
        m = dict(shared)
        m.update(per_core[k])
        in_maps.append(m)

    res = bass_utils.run_bass_kernel_spmd(nc, in_maps, core_ids=list(range(N_CORES)))
    out = np.concatenate([res.results[k]["y"] for k in range(N_CORES)], axis=0)
    return out.astype(np.float32)
